# revision 9
# baseline (speedup 1.0000x reference)
"""Clockwork RNN (CWRNN) Trainium2 Bass kernel.

Problem (hardcoded from spec): B=512, T=192, DX=32, DY=4 heads, DH1=256,
DH2=512, update rates (1,2,4) over hidden blocks of (128,64,64) units.

Sharding: 8 cores = 4 heads x 2 batch-halves (B_core=256). Heads are fully
independent; batch is data-parallel.

Per-core dataflow (everything transposed: units on partitions, batch free):
  - state h kept as SBUF tile [128, 2, 256]: h[p, c, b] = h_unit(c*128+p).
  - per step j: cand^T accumulated in PSUM via matmuls
      wxb^T @ [x_t; 1]  (K=33, bias folded in via a ones row)
    + w_h[128:256]^T @ h1  and  w_h[0:128]^T @ h0  (K=128 each),
    then tanh on the scalar engine writes updated units back into h.
    Clock gating = only computing/writing the first k units (partition rows),
    so the schedule's "where" is free.
  - matmul dtype is float32r (fp32 with 11-bit mantissa, full-rate PE) or
    bf16; fp32 runs at quarter rate on the PE.
  - final MLP: hid = relu(W1^T h + b1), y = W2 . hid (+ b2 on host).
"""

import numpy as np
import ml_dtypes

import concourse.bass as bass
import concourse.mybir as mybir
import concourse.tile as tile
from concourse import bacc
from concourse.bass_utils import run_bass_kernel_spmd

F32 = mybir.dt.float32
TANH = mybir.ActivationFunctionType.Tanh
RELU = mybir.ActivationFunctionType.Relu

B, T, DX, DY, DH1, DH2 = 512, 192, 32, 4, 256, 512
KX = DX + 1          # w_x rows + folded bias row
BC = B // 2          # batch per core (256)
TW = 16              # timesteps per x window DMA

# matmul operand dtype: "f32r" (11-bit mantissa, ~5e-4 end-to-end err),
# "bf16" (~5e-3), or "f32" (exact but 4x slower PE)
MM_DT = "f32r"
# batch chunks per core (2 shortens the serial chain but needs full-rate
# matmuls at moving dim 128, which f32r does not have)
NCH = {"f32r": 1, "bf16": 2, "f32": 2}
# merge the type-3 m0+m1 tanh into one activation
MERGE3 = True

_nc_cache = {}


def _mm_dtype(name):
    return {"f32r": mybir.dt.float32r, "bf16": mybir.dt.bfloat16,
            "f32": mybir.dt.float32}[name]


def _round_f32r(a):
    v = np.ascontiguousarray(a, np.float32).view(np.uint32)
    r = ((v + np.uint32(0x800)) & np.uint32(0xFFFFF000))
    return r.view(np.float32).copy()


def _host_cast(a, dt_name):
    if dt_name == "f32r":
        return _round_f32r(a)
    if dt_name == "bf16":
        return np.asarray(a, np.float32).astype(ml_dtypes.bfloat16)
    return np.ascontiguousarray(a, np.float32)


def _step_type(j):
    if (j + 1) % 4 == 0:
        return 3
    if (j + 1) % 2 == 0:
        return 2
    return 1


def _feat_mask():
    feat = np.zeros((T, DX), np.float32)
    for j in range(T):
        n = {1: 16, 2: 24, 3: 32}[_step_type(j)]
        feat[j, :n] = 1.0
    return feat


def build_nc(mm_dt=MM_DT, nch=None, merge3=MERGE3):
    DT = _mm_dtype(mm_dt)
    if nch is None:
        nch = NCH[mm_dt]
    cb = BC // nch
    nc = bacc.Bacc("TRN2", target_bir_lowering=False, debug=False)
    xt_d = nc.dram_tensor("xt", [KX, T, BC], DT, kind="ExternalInput")
    wh_d = nc.dram_tensor("wh", [DH1, DH1], DT, kind="ExternalInput")
    wxb_d = nc.dram_tensor("wxb", [KX, DH1], DT, kind="ExternalInput")
    w1_d = nc.dram_tensor("w1", [DH1, DH2], DT, kind="ExternalInput")
    b1_d = nc.dram_tensor("b1t", [128, 4], F32, kind="ExternalInput")
    w2_d = nc.dram_tensor("w2t", [128, 4], DT, kind="ExternalInput")
    y_d = nc.dram_tensor("y", [1, BC], F32, kind="ExternalOutput")

    with tile.TileContext(nc) as tc:
        with (
            tc.tile_pool(name="const", bufs=1) as cpool,
            tc.tile_pool(name="state", bufs=1) as spool,
            tc.tile_pool(name="xw", bufs=3) as xpool,
            tc.tile_pool(name="ps", bufs=2, space=bass.MemorySpace.PSUM) as pspool,
            tc.tile_pool(name="psm1", bufs=1, space=bass.MemorySpace.PSUM) as psm1pool,
            tc.tile_pool(name="ps2", bufs=1, space=bass.MemorySpace.PSUM) as ps2pool,
            tc.tile_pool(name="misc", bufs=1) as mpool,
        ):
            wh_sb = cpool.tile([128, 2, DH1], DT, tag="wh")
            for c in range(2):
                nc.sync.dma_start(wh_sb[:, c, :], wh_d[c * 128:(c + 1) * 128, :])
            wxb_sb = cpool.tile([KX, DH1], DT, tag="wxb")
            nc.sync.dma_start(wxb_sb[:], wxb_d[:])
            w1_sb = cpool.tile([128, 2, DH2], DT, tag="w1")
            for c in range(2):
                nc.sync.dma_start(w1_sb[:, c, :], w1_d[c * 128:(c + 1) * 128, :])
            b1_sb = cpool.tile([128, 4], F32, tag="b1")
            nc.sync.dma_start(b1_sb[:], b1_d[:])
            w2_sb = cpool.tile([128, 4], DT, tag="w2")
            nc.sync.dma_start(w2_sb[:], w2_d[:])

            # h is never zero-initialized: matmuls reading a still-unwritten
            # block of h are skipped (j=0,1) or K-narrowed (j=2,3), which is
            # the same math since those units are zero.
            h = spool.tile([128, 2, BC], DT, tag="h")

            xw = None
            for j in range(T):
                r = j % TW
                if r == 0:
                    xw = xpool.tile([KX, TW, BC], DT, tag="xw")
                    nc.sync.dma_start(xw[:], xt_d[:, j:j + TW, :])
                typ = _step_type(j)

                def emit_group(mms):
                    # one accumulation group per PSUM bank (start clears
                    # has_written bank-wide; each region's first matmul
                    # overwrites, later ones accumulate). The tanh reading
                    # a bank always depends on that bank's stop matmul, so
                    # ACT never reads while PE writes the same bank.
                    for i, (o, lt, rh) in enumerate(mms):
                        nc.tensor.matmul(o, lt, rh, start=(i == 0),
                                         stop=(i == len(mms) - 1))

                crit_acts, late_acts = [], []
                for ch in range(nch):
                    cs = slice(ch * cb, (ch + 1) * cb)
                    ps = pspool.tile([128, 2, cb], F32, tag=f"ps{ch}")

                    def c1(cols):
                        # h units 192:256 are first written at j=3; before
                        # that, narrow the h1 contraction to K=64 (the rest
                        # would multiply uninitialized zeros)
                        if j <= 3:
                            return (wh_sb[0:64, 1, cols], h[0:64, 1, cs])
                        return (wh_sb[:, 1, cols], h[:, 1, cs])

                    # m0 (units 0:128): wx (+bias), then h1, then fresh h0 last
                    mm0 = [(ps[:, 0, :], wxb_sb[:, 0:128], xw[:, r, cs])]
                    if j >= 2:
                        mm0.append((ps[:, 0, :], *c1(slice(0, 128))))
                    if j >= 1:
                        mm0.append((ps[:, 0, :], wh_sb[:, 0, 0:128], h[:, 0, cs]))

                    if typ == 3:
                        mm1 = [(ps[:, 1, :], wxb_sb[:, 128:256], xw[:, r, cs])]
                        if j >= 2:
                            mm1.append((ps[:, 1, :], *c1(slice(128, 256))))
                        if j >= 1:
                            mm1.append((ps[:, 1, :], wh_sb[:, 0, 128:256], h[:, 0, cs]))
                        emit_group(mm0 + mm1)
                        if merge3:
                            # m1 shares the bank; one merged tanh after the stop
                            crit_acts.append((h[:, 0:2, cs], ps[:, 0:2, :]))
                        else:
                            crit_acts.append((h[:, 0, cs], ps[:, 0, :]))
                            late_acts.append((h[:, 1, cs], ps[:, 1, :]))
                    elif typ == 2:
                        # partial m1 (64 units) in its own bank so the critical
                        # m0 tanh is not held back behind the m1 matmuls.
                        # All matmuls reading h are emitted before the tanh
                        # that overwrites it (WAR keeps old values).
                        pm1 = psm1pool.tile([64, cb], F32, tag=f"t2m1_{ch}")
                        emit_group(mm0)
                        mm1 = [(pm1[:], wxb_sb[:, 128:192], xw[:, r, cs])]
                        if j >= 2:
                            mm1.append((pm1[:], *c1(slice(128, 192))))
                        if j >= 1:
                            mm1.append((pm1[:], wh_sb[:, 0, 128:192], h[:, 0, cs]))
                        emit_group(mm1)
                        crit_acts.append((h[:, 0, cs], ps[:, 0, :]))
                        late_acts.append((h[0:64, 1, cs], pm1[:]))
                    else:
                        emit_group(mm0)
                        crit_acts.append((h[:, 0, cs], ps[:, 0, :]))
                # all chunks' critical (m0/merged) tanhs go first in the ACT
                # FIFO; the m1 tanhs are only needed one step later
                for o, i_ in crit_acts + late_acts:
                    nc.scalar.activation(o, i_, TANH)

            # output MLP: hid = relu(W1^T h + b1); y = W2 . hid
            hid = mpool.tile([128, 4, BC], DT, tag="hid")
            for m in range(4):
                ms = slice(m * 128, (m + 1) * 128)
                pm = ps2pool.tile([128, BC], F32, tag="mlp")
                nc.tensor.matmul(pm[:], w1_sb[:, 0, ms], h[:, 0, :], start=True, stop=False)
                nc.tensor.matmul(pm[:], w1_sb[:, 1, ms], h[:, 1, :], start=False, stop=True)
                nc.scalar.activation(hid[:, m, :], pm[:], RELU, bias=b1_sb[:, m:m + 1])
            yp = ps2pool.tile([1, BC], F32, tag="yp")
            for m in range(4):
                nc.tensor.matmul(yp[:], w2_sb[:, m:m + 1], hid[:, m, :],
                                 start=(m == 0), stop=(m == 3))
            ysb = mpool.tile([1, BC], F32, tag="ysb")
            nc.vector.tensor_copy(ysb[:], yp[:])
            nc.sync.dma_start(y_d[:], ysb[:])

    nc.compile()
    return nc


def make_in_maps(x, w_x, w_h, b, W1, b1, W2, mm_dt=MM_DT):
    feat = _feat_mask()
    xm = np.asarray(x, np.float32) * feat[None, :, :]   # [B, T, DX]
    xt = np.empty((KX, T, B), np.float32)
    xt[:DX] = xm.transpose(2, 1, 0)
    xt[DX] = 1.0
    xt = _host_cast(xt, mm_dt)
    in_maps = []
    for core in range(8):
        h_idx, s = divmod(core, 2)
        wxb = np.concatenate([np.asarray(w_x[h_idx], np.float32),
                              np.asarray(b[h_idx], np.float32)[None, :]], axis=0)
        in_maps.append({
            "xt": np.ascontiguousarray(xt[:, :, s * BC:(s + 1) * BC]),
            "wh": _host_cast(w_h[h_idx], mm_dt),
            "wxb": _host_cast(wxb, mm_dt),
            "w1": _host_cast(W1[h_idx], mm_dt),
            "b1t": np.ascontiguousarray(np.asarray(b1[h_idx], np.float32).reshape(4, 128).T),
            "w2t": _host_cast(np.asarray(W2[h_idx], np.float32).reshape(4, 128).T, mm_dt),
        })
    return in_maps


def kernel(x, w_x, w_h, b, W1, b1, W2, b2):
    if "nc" not in _nc_cache:
        _nc_cache["nc"] = build_nc()
    nc = _nc_cache["nc"]
    in_maps = make_in_maps(x, w_x, w_h, b, W1, b1, W2)
    res = run_bass_kernel_spmd(nc, in_maps, core_ids=list(range(8)))
    b2 = np.asarray(b2, np.float32)
    y = np.empty((B, DY), np.float32)
    for core in range(8):
        h_idx, s = divmod(core, 2)
        y[s * BC:(s + 1) * BC, h_idx] = res.results[core]["y"][0] + b2[h_idx]
    return y
